# revision 1
# baseline (speedup 1.0000x reference)
"""Trainium2 Bass kernel for nn_AttentionBlock (B=4, H=W=64, C=256, D=32).

Sharding: 8 shards = 4 samples x 2 query-halves. Each core gets the full
sample's rows (reordered so its 2048 query rows come first), computes K/V
for all 4096 keys, and attention + output projection + residual for its
2048 queries. No collectives needed.

Self-contained: hardcodes shapes, imports only /opt/trn_rl_repo concourse.
"""

import sys

if "/opt/trn_rl_repo" not in sys.path:
    sys.path.insert(0, "/opt/trn_rl_repo")

import numpy as np
import ml_dtypes

BF16 = ml_dtypes.bfloat16

# Problem constants
B, HH, WW, C = 4, 64, 64, 256
D = 32
N = HH * WW          # 4096 keys per sample
NQ = N // 2          # 2048 queries per core
NCORES = 8
KC = N // 128        # 32 key chunks
QB = NQ // 128       # 16 query blocks per core

_compiled_cache = {}


def _build(use_bias: bool):
    from contextlib import ExitStack
    from concourse import bacc, tile, mybir, masks

    f32 = mybir.dt.float32
    bf = mybir.dt.bfloat16

    nc = bacc.Bacc("TRN2", target_bir_lowering=False, debug=False, num_devices=NCORES)

    x16_d = nc.dram_tensor("x16", [N, C], bf, kind="ExternalInput")
    xq32_d = nc.dram_tensor("xq32", [NQ, C], f32, kind="ExternalInput")
    wqa_d = nc.dram_tensor("wqa_rep", [257, 128], bf, kind="ExternalInput")
    wka_d = nc.dram_tensor("wka_rep", [257, 128], bf, kind="ExternalInput")
    wva_d = nc.dram_tensor("wva", [257, 256], bf, kind="ExternalInput")
    woa_d = nc.dram_tensor("woa", [257, 256], bf, kind="ExternalInput")
    out_d = nc.dram_tensor("out", [NQ, C], f32, kind="ExternalOutput")

    Exp = mybir.ActivationFunctionType.Exp
    Add = mybir.AluOpType.add
    Mult = mybir.AluOpType.mult

    with tile.TileContext(nc) as tc:
        with ExitStack() as ctx:
            const = ctx.enter_context(tc.tile_pool(name="const", bufs=1))
            big = ctx.enter_context(tc.tile_pool(name="big", bufs=1))
            expp = ctx.enter_context(tc.tile_pool(name="expp", bufs=6))
            small = ctx.enter_context(tc.tile_pool(name="small", bufs=2))
            ps_s = ctx.enter_context(tc.tile_pool(name="ps_s", bufs=2, space="PSUM"))
            ps_att = ctx.enter_context(tc.tile_pool(name="ps_att", bufs=2, space="PSUM"))
            ps_misc = ctx.enter_context(tc.tile_pool(name="ps_misc", bufs=2, space="PSUM"))

            # ---- constants & weights ----
            ident = const.tile([128, 128], bf, tag="ident")
            masks.make_identity(nc, ident[:])
            ones_row = const.tile([1, 512], bf, tag="ones_row")
            nc.gpsimd.memset(ones_row[:], 1.0)

            wq0 = const.tile([128, 128], bf, tag="wq0")
            wq1 = const.tile([128, 128], bf, tag="wq1")
            wk0 = const.tile([128, 128], bf, tag="wk0")
            wk1 = const.tile([128, 128], bf, tag="wk1")
            wv0 = const.tile([128, 256], bf, tag="wv0")
            wv1 = const.tile([128, 256], bf, tag="wv1")
            wo0 = const.tile([128, 256], bf, tag="wo0")
            wo1 = const.tile([128, 256], bf, tag="wo1")
            nc.sync.dma_start(out=wq0[:], in_=wqa_d[0:128, :])
            nc.sync.dma_start(out=wq1[:], in_=wqa_d[128:256, :])
            nc.sync.dma_start(out=wk0[:], in_=wka_d[0:128, :])
            nc.sync.dma_start(out=wk1[:], in_=wka_d[128:256, :])
            nc.sync.dma_start(out=wv0[:], in_=wva_d[0:128, :])
            nc.sync.dma_start(out=wv1[:], in_=wva_d[128:256, :])
            nc.sync.dma_start(out=wo0[:], in_=woa_d[0:128, :])
            nc.sync.dma_start(out=wo1[:], in_=woa_d[128:256, :])
            if use_bias:
                wqb = const.tile([1, 128], bf, tag="wqb")
                wkb = const.tile([1, 128], bf, tag="wkb")
                wvb = const.tile([1, 256], bf, tag="wvb")
                wob = const.tile([1, 256], bf, tag="wob")
                nc.sync.dma_start(out=wqb[:], in_=wqa_d[256:257, :])
                nc.sync.dma_start(out=wkb[:], in_=wka_d[256:257, :])
                nc.sync.dma_start(out=wvb[:], in_=wva_d[256:257, :])
                nc.sync.dma_start(out=wob[:], in_=woa_d[256:257, :])

            # ---- phase A: x -> xT (channel-major), via identity matmuls ----
            # x arrives as [4096, 256] bf16; load as [128, 32, 256] via 8 large
            # strided DMAs (issue cost is ~0.6us per dma_start, so few + big),
            # split across two issuing engines.
            xbig = big.tile([128, KC, 256], bf, tag="xbig")
            x_r = x16_d[:].rearrange("(t p) c -> p t c", p=128)
            for d in range(32):
                nc.sync.dma_start(out=xbig[:, d : d + 1, :], in_=x_r[:, d : d + 1, :])
            xT = big.tile([128, 2, N], bf, tag="xT")  # [:, h, :]: channels 128h..128h+127
            for t in range(16):
                ta, tb = 2 * t, 2 * t + 1
                pt = ps_s.tile([128, 1024], f32, tag="s")
                nc.tensor.matmul(pt[:, 0:128], xbig[:, ta, 0:128], ident[:], start=True, stop=True)
                nc.tensor.matmul(pt[:, 128:256], xbig[:, tb, 0:128], ident[:], start=True, stop=True)
                nc.tensor.matmul(pt[:, 512:640], xbig[:, ta, 128:256], ident[:], start=True, stop=True)
                nc.tensor.matmul(pt[:, 640:768], xbig[:, tb, 128:256], ident[:], start=True, stop=True)
                nc.vector.tensor_copy(xT[:, 0, 256 * t : 256 * t + 256], pt[:, 0:256])
                nc.vector.tensor_copy(xT[:, 1, 256 * t : 256 * t + 256], pt[:, 512:768])

            # ---- phase B: qT/kT (d on partitions 0..31) and V ----
            qT = big.tile([32, NQ], bf, tag="qT")
            kT = big.tile([32, N], bf, tag="kT")
            for s in range(NQ // 512):
                pq = ps_s.tile([128, 1024], f32, tag="s")
                nc.tensor.matmul(pq[0:32, 0:512], wq0[:, 0:32], xT[:, 0, 512 * s : 512 * s + 512], start=True, stop=False)
                nc.tensor.matmul(pq[0:32, 0:512], wq1[:, 0:32], xT[:, 1, 512 * s : 512 * s + 512], start=False, stop=not use_bias)
                if use_bias:
                    nc.tensor.matmul(pq[0:32, 0:512], wqb[:, 0:32], ones_row[:, 0:512], start=False, stop=True)
                nc.vector.tensor_copy(qT[:, 512 * s : 512 * s + 512], pq[0:32, 0:512])
            for s in range(N // 512):
                pk = ps_s.tile([128, 1024], f32, tag="s")
                nc.tensor.matmul(pk[0:32, 0:512], wk0[:, 0:32], xT[:, 0, 512 * s : 512 * s + 512], start=True, stop=False)
                nc.tensor.matmul(pk[0:32, 0:512], wk1[:, 0:32], xT[:, 1, 512 * s : 512 * s + 512], start=False, stop=not use_bias)
                if use_bias:
                    nc.tensor.matmul(pk[0:32, 0:512], wkb[:, 0:32], ones_row[:, 0:512], start=False, stop=True)
                nc.vector.tensor_copy(kT[:, 512 * s : 512 * s + 512], pk[0:32, 0:512])

            # V rows (keys) with a ones column at 256 for the softmax denominator
            vsb = big.tile([128, KC, 260], bf, tag="vsb")
            nc.vector.memset(vsb[:, :, 256:257], 1.0)
            for m in range(KC):
                pv = ps_misc.tile([128, 256], f32, tag="m")
                nc.tensor.matmul(pv[:], xT[:, 0, 128 * m : 128 * m + 128], wv0[:], start=True, stop=False)
                nc.tensor.matmul(pv[:], xT[:, 1, 128 * m : 128 * m + 128], wv1[:], start=False, stop=not use_bias)
                if use_bias:
                    nc.tensor.matmul(pv[:], ones_row[:, 0:128], wvb[:], start=False, stop=True)
                nc.vector.tensor_copy(vsb[:, m, 0:256], pv[:])

            # ---- phase C: software-pipelined S -> exp -> attend, per 128-query block ----
            def epilogue(qb, pa):
                rec = small.tile([128, 1], f32, tag="rec")
                nc.vector.reciprocal(rec[:], pa[:, 256:257])
                at = small.tile([128, 256], bf, tag="attn")
                nc.vector.tensor_scalar(at[:], pa[:, 0:256], rec[:], None, Mult)
                ptr = ps_misc.tile([128, 256], f32, tag="m")
                nc.tensor.matmul(ptr[:, 0:128], at[:, 0:128], ident[:], start=True, stop=True)
                nc.tensor.matmul(ptr[:, 128:256], at[:, 128:256], ident[:], start=True, stop=True)
                aT = small.tile([128, 256], bf, tag="aT")
                nc.vector.tensor_copy(aT[:], ptr[:])
                po = ps_misc.tile([128, 256], f32, tag="m")
                nc.tensor.matmul(po[:], aT[:, 0:128], wo0[:], start=True, stop=False)
                nc.tensor.matmul(po[:], aT[:, 128:256], wo1[:], start=False, stop=not use_bias)
                if use_bias:
                    nc.tensor.matmul(po[:], ones_row[:, 0:128], wob[:], start=False, stop=True)
                xq = small.tile([128, 256], f32, tag="xq", bufs=3)
                nc.sync.dma_start(out=xq[:], in_=xq32_d[128 * qb : 128 * qb + 128, :])
                ot = small.tile([128, 256], f32, tag="ot", bufs=3)
                nc.vector.tensor_tensor(ot[:], po[:], xq[:], Add)
                nc.sync.dma_start(out=out_d[128 * qb : 128 * qb + 128, :], in_=ot[:])

            # Process query blocks in PAIRS (256 query columns per S matmul):
            # each group g covers qblocks 2g, 2g+1 in 8 steps of 4 key chunks.
            # S psum tile [128, 1024] holds 4 chunks x 256 q; one EXP covers it.
            pa_tiles = {}
            prev = None  # (et, g, t)
            for s in range(8 * (QB // 2) + 1):
                if s < 8 * (QB // 2):
                    g, t = divmod(s, 8)
                    if t == 0:
                        pa_tiles[2 * g] = ps_att.tile([128, 260], f32, tag="a", name=f"pa{2 * g}")
                        pa_tiles[2 * g + 1] = ps_att.tile([128, 260], f32, tag="a", name=f"pa{2 * g + 1}")
                    pst = ps_s.tile([128, 1024], f32, tag="s")
                    for cc in range(4):
                        m = 4 * t + cc
                        nc.tensor.matmul(
                            pst[:, 256 * cc : 256 * cc + 256],
                            kT[:, 128 * m : 128 * m + 128],
                            qT[:, 256 * g : 256 * g + 256],
                            start=True,
                            stop=True,
                        )
                # attend with previous step's exp tile (keeps PE busy during exp)
                if prev is not None:
                    et_p, g_p, t_p = prev
                    for cc in range(4):
                        m = 4 * t_p + cc
                        for h in range(2):
                            nc.tensor.matmul(
                                pa_tiles[2 * g_p + h][:, 0:257],
                                et_p[:, 256 * cc + 128 * h : 256 * cc + 128 * h + 128],
                                vsb[:, m, 0:257],
                                start=(m == 0),
                                stop=(m == KC - 1),
                            )
                    if t_p == 7:
                        for h in range(2):
                            epilogue(2 * g_p + h, pa_tiles[2 * g_p + h])
                            del pa_tiles[2 * g_p + h]
                if s < 8 * (QB // 2):
                    et = expp.tile([128, 1024], bf, tag="e")
                    nc.scalar.activation(et[:], pst[:], Exp)
                    prev = (et, g, t)

    nc.compile()
    return nc


def _get_compiled(use_bias: bool):
    key = bool(use_bias)
    if key not in _compiled_cache:
        _compiled_cache[key] = _build(use_bias)
    return _compiled_cache[key]


def _prep(x, wq, bq, wk, bk, wv, bv, wo, bo):
    xf = np.ascontiguousarray(np.asarray(x, dtype=np.float32)).reshape(B, N, C)
    wq = np.asarray(wq, np.float32)
    bq = np.asarray(bq, np.float32)
    wk = np.asarray(wk, np.float32)
    bk = np.asarray(bk, np.float32)
    wv = np.asarray(wv, np.float32)
    bv = np.asarray(bv, np.float32)
    wo = np.asarray(wo, np.float32)
    bo = np.asarray(bo, np.float32)

    use_bias = not (
        np.all(bq == 0) and np.all(bk == 0) and np.all(bv == 0) and np.all(bo == 0)
    )

    scale = np.float32(1.0 / np.sqrt(np.float32(D)))
    wqa = np.concatenate([wq, bq[None, :]], 0) * scale  # fold softmax scale into q
    wka = np.concatenate([wk, bk[None, :]], 0)
    wqa_rep = np.ascontiguousarray(np.tile(wqa, (1, 4))).astype(BF16)  # [257, 128]
    wka_rep = np.ascontiguousarray(np.tile(wka, (1, 4))).astype(BF16)
    wva = np.concatenate([wv, bv[None, :]], 0).astype(BF16)  # [257, 256]
    woa = np.concatenate([wo, bo[None, :]], 0).astype(BF16)

    in_maps = []
    for core in range(NCORES):
        b, h = divmod(core, 2)
        if h == 0:
            xo = xf[b]
        else:
            xo = np.concatenate([xf[b, NQ:], xf[b, :NQ]], 0)
        in_maps.append(
            {
                "x16": xo.astype(BF16),
                "xq32": np.ascontiguousarray(xo[:NQ]),
                "wqa_rep": wqa_rep,
                "wka_rep": wka_rep,
                "wva": wva,
                "woa": woa,
            }
        )
    return in_maps, use_bias


def _gather(results):
    out = np.empty((B, N, C), np.float32)
    for core in range(NCORES):
        b, h = divmod(core, 2)
        out[b, NQ * h : NQ * (h + 1)] = results[core]["out"]
    return out.reshape(B, HH, WW, C)


def kernel(x, wq, bq, wk, bk, wv, bv, wo, bo):
    from concourse.bass_utils import run_bass_kernel_spmd

    in_maps, use_bias = _prep(x, wq, bq, wk, bk, wv, bv, wo, bo)
    nc = _get_compiled(use_bias)
    res = run_bass_kernel_spmd(nc, in_maps, core_ids=list(range(NCORES)))
    return _gather(res.results)


def _ensure_ntff_hook():
    """The agent image's antenv stub lacks axon_hooks; synthesize it so
    run_bass_kernel_spmd(trace=True) can NTFF-profile via libaxon_pjrt."""
    import types

    try:
        from antenv.axon_hooks import get_axon_ntff_profile_hook  # noqa: F401
        return
    except ImportError:
        pass
    import antenv
    from trn_agent_boot.trn_boot import _ntff_profile_via_ctypes

    mod = types.ModuleType("antenv.axon_hooks")
    state = {"h": _ntff_profile_via_ctypes("/opt/axon/libaxon_pjrt.so")}
    mod.get_axon_ntff_profile_hook = lambda: state["h"]
    mod.set_axon_ntff_profile_hook = lambda h: state.__setitem__("h", h)
    sys.modules["antenv.axon_hooks"] = mod
    antenv.axon_hooks = mod


def run_traced(inputs, **kw):
    """For test.py: run with NTFF profiling; returns (output, BassKernelResults)."""
    from concourse.bass_utils import run_bass_kernel_spmd

    _ensure_ntff_hook()

    in_maps, use_bias = _prep(**inputs)
    nc = _get_compiled(use_bias)
    res = run_bass_kernel_spmd(nc, in_maps, core_ids=list(range(NCORES)), trace=True, **kw)
    return _gather(res.results), res



# revision 6
# speedup vs baseline: 1.2704x; 1.2704x over previous
"""Trainium2 Bass kernel for nn_AttentionBlock (B=4, H=W=64, C=256, D=32).

Sharding: 8 shards = 4 samples x 2 query-halves. Each core gets the full
sample's rows (reordered so its 2048 query rows come first), computes K for
all 4096 keys, and attention for its 2048 queries. No collectives.

v2 algorithm (projection folding + fp8):
  out = x + (1/d) * (G^T @ W2),  W2 = 32 * wv @ wo   (host precompute)
  G[c,q] = sum_k x8[k,c] * E8[k,q]   (fp8 DoubleRow matmuls, contraction 256)
  E8 = fp8e5m2(exp(S - 2)),  S = K Q^T scores  [keys, queries]
  d[q] = sum_k E8[k,q]       (col-packed ones matmuls + transpose matmul)
exp is computed two ways in parallel: ACT true exp -> e5m2, and DVE
integer bit-trick (Schraudolph in e5m2 space: bits = 5.7708*(S-2)+60.3).
S matmuls (contraction D=32) use 4-way PE row-tiling via tile_position.

Self-contained: hardcodes shapes, imports only /opt/trn_rl_repo concourse.
"""

import sys

if "/opt/trn_rl_repo" not in sys.path:
    sys.path.insert(0, "/opt/trn_rl_repo")

import numpy as np
import ml_dtypes

BF16 = ml_dtypes.bfloat16
E4M3 = ml_dtypes.float8_e4m3
E5M2 = ml_dtypes.float8_e5m2

# Problem constants
B, HH, WW, C = 4, 64, 64, 256
D = 32
N = HH * WW           # 4096 keys per sample
NQ = N // 2           # 2048 queries per core
NCORES = 8
KC = N // 128         # 32 key chunks
NG = NQ // 512        # 4 query groups of 512 per core
NSTEP = 8             # 4-chunk steps per query group (32 chunks / 4)

C0 = 2.0              # exp shift: weights = exp(S - C0), cancels in softmax
EXP_A = 5.770780      # 4 * log2(e)
EXP_B = 60.0 + 0.3 - EXP_A * C0  # e5m2 bias 60, +0.3 truncation recenter

_compiled_cache = {}


def _build():
    from contextlib import ExitStack
    from concourse import bacc, tile, mybir, masks

    f32 = mybir.dt.float32
    bf = mybir.dt.bfloat16
    fp8e4 = mybir.dt.float8e4
    fp8e5 = mybir.dt.float8e5
    u8 = mybir.dt.uint8

    nc = bacc.Bacc("TRN2", target_bir_lowering=False, debug=False, num_devices=NCORES)

    x16_d = nc.dram_tensor("x16", [N, C], bf, kind="ExternalInput")
    x8_d = nc.dram_tensor("x8", [N, C], fp8e4, kind="ExternalInput")
    xq32_d = nc.dram_tensor("xq32", [NQ, C], f32, kind="ExternalInput")
    wqa_d = nc.dram_tensor("wqa_rep", [256, 128], bf, kind="ExternalInput")
    wka_d = nc.dram_tensor("wka_rep", [256, 128], bf, kind="ExternalInput")
    bq_d = nc.dram_tensor("bq_col", [128, 1], f32, kind="ExternalInput")
    bk_d = nc.dram_tensor("bk_col", [128, 1], f32, kind="ExternalInput")
    w2_d = nc.dram_tensor("w2", [256, 256], bf, kind="ExternalInput")
    out_d = nc.dram_tensor("out", [NQ, C], f32, kind="ExternalOutput")

    Exp = mybir.ActivationFunctionType.Exp
    Add = mybir.AluOpType.add
    Mult = mybir.AluOpType.mult
    DR = mybir.MatmulPerfMode.DoubleRow

    with tile.TileContext(nc) as tc:
        with ExitStack() as ctx:
            const = ctx.enter_context(tc.tile_pool(name="const", bufs=1))
            big = ctx.enter_context(tc.tile_pool(name="big", bufs=1))
            expp = ctx.enter_context(tc.tile_pool(name="expp", bufs=6))
            small = ctx.enter_context(tc.tile_pool(name="small", bufs=2))
            ps_s = ctx.enter_context(tc.tile_pool(name="ps_s", bufs=2, space="PSUM"))
            ps_g = ctx.enter_context(tc.tile_pool(name="ps_g", bufs=1, space="PSUM"))
            ps_d = ctx.enter_context(tc.tile_pool(name="ps_d", bufs=1, space="PSUM"))
            ps_e = ctx.enter_context(tc.tile_pool(name="ps_e", bufs=1, space="PSUM"))

            # ---- constants & weights ----
            ident = const.tile([128, 128], bf, tag="ident")
            masks.make_identity(nc, ident[:])
            ones8 = const.tile([128, 32], fp8e5, tag="ones8")
            nc.gpsimd.memset(ones8[:], 1.0)
            ones1 = const.tile([128, 1], bf, tag="ones1")
            nc.gpsimd.memset(ones1[:], 1.0)
            negc0 = const.tile([128, 1], f32, tag="negc0")
            nc.gpsimd.memset(negc0[:], -C0)

            wq0 = const.tile([128, 128], bf, tag="wq0")
            wq1 = const.tile([128, 128], bf, tag="wq1")
            wk0 = const.tile([128, 128], bf, tag="wk0")
            wk1 = const.tile([128, 128], bf, tag="wk1")
            w2sb = const.tile([128, 2, 256], bf, tag="w2sb")
            bqc = const.tile([128, 1], f32, tag="bqc")
            bkc = const.tile([128, 1], f32, tag="bkc")
            nc.sync.dma_start(out=wq0[:], in_=wqa_d[0:128, :])
            nc.sync.dma_start(out=wq1[:], in_=wqa_d[128:256, :])
            nc.sync.dma_start(out=wk0[:], in_=wka_d[0:128, :])
            nc.sync.dma_start(out=wk1[:], in_=wka_d[128:256, :])
            nc.sync.dma_start(out=w2sb[:, 0, :], in_=w2_d[0:128, :])
            nc.sync.dma_start(out=w2sb[:, 1, :], in_=w2_d[128:256, :])
            nc.sync.dma_start(out=bqc[:], in_=bq_d[:])
            nc.sync.dma_start(out=bkc[:], in_=bk_d[:])

            # ---- phase A: x -> xT (channel-major), via identity matmuls ----
            xbig = big.tile([128, KC, 256], bf, tag="xbig")
            x_r = x16_d[:].rearrange("(t p) c -> p t c", p=128)
            for d in range(KC):
                nc.sync.dma_start(out=xbig[:, d : d + 1, :], in_=x_r[:, d : d + 1, :])
            x8sb = big.tile([128, KC, 256], fp8e4, tag="x8sb")
            x8_r = x8_d[:].rearrange("(t p) c -> p t c", p=128)
            for d in range(KC // 2):
                nc.scalar.dma_start(
                    out=x8sb[:, 2 * d : 2 * d + 2, :], in_=x8_r[:, 2 * d : 2 * d + 2, :]
                )

            xT = big.tile([128, 2, N], bf, tag="xT")  # [:, h, :]: channels 128h..128h+127
            for t in range(16):
                ta, tb = 2 * t, 2 * t + 1
                pt = ps_s.tile([128, 2, 512], f32, tag="s")
                nc.tensor.matmul(pt[:, 0, 0:128], xbig[:, ta, 0:128], ident[:], start=True, stop=True)
                nc.tensor.matmul(pt[:, 0, 128:256], xbig[:, tb, 0:128], ident[:], start=True, stop=True)
                nc.tensor.matmul(pt[:, 1, 0:128], xbig[:, ta, 128:256], ident[:], start=True, stop=True)
                nc.tensor.matmul(pt[:, 1, 128:256], xbig[:, tb, 128:256], ident[:], start=True, stop=True)
                nc.vector.tensor_copy(xT[:, 0, 256 * t : 256 * t + 256], pt[:, 0, 0:256])
                nc.vector.tensor_copy(xT[:, 1, 256 * t : 256 * t + 256], pt[:, 1, 0:256])

            # ---- phase B: qT/kT replicated x4 along partitions (d = 32 each) ----
            qT = big.tile([128, NQ], bf, tag="qT")
            kT = big.tile([128, N], bf, tag="kT")
            for s in range(NQ // 512):
                pq = ps_s.tile([128, 2, 512], f32, tag="s")
                nc.tensor.matmul(pq[:, 0, :], wq0[:], xT[:, 0, 512 * s : 512 * s + 512], start=True, stop=False)
                nc.tensor.matmul(pq[:, 0, :], wq1[:], xT[:, 1, 512 * s : 512 * s + 512], start=False, stop=True)
                nc.vector.tensor_scalar(qT[:, 512 * s : 512 * s + 512], pq[:, 0, :], bqc[:], None, Add)
            for s in range(N // 512):
                pk = ps_s.tile([128, 2, 512], f32, tag="s")
                nc.tensor.matmul(pk[:, 0, :], wk0[:], xT[:, 0, 512 * s : 512 * s + 512], start=True, stop=False)
                nc.tensor.matmul(pk[:, 0, :], wk1[:], xT[:, 1, 512 * s : 512 * s + 512], start=False, stop=True)
                nc.vector.tensor_scalar(kT[:, 512 * s : 512 * s + 512], pk[:, 0, :], bkc[:], None, Add)

            # ---- phase C: flat pipeline over 32 steps of 4 key chunks ----
            # step s (produce): S matmuls 4-way row-packed + exp (ACT pair A, DVE pair B)
            # step s (consume s-2): G DoubleRow matmuls + denom col-packed matmuls
            # group boundary: rec + output projection + residual + store
            sts = {}   # step -> (sA, sB) psum tiles
            ets = {}   # step -> (etA, etB) fp8 tiles
            gtile = {}  # group -> G psum tile
            dtile = {}  # group -> denom psum tile

            def produce(s):
                g, t = divmod(s, NSTEP)
                if t == 0:
                    gtile[g] = ps_g.tile([128, 2, 512], f32, tag="g", name=f"g{g}")
                    dtile[g] = ps_d.tile([128, 512], f32, tag="d", name=f"d{g}")
                sA = ps_s.tile([128, 2, 512], f32, tag="s", name=f"sA{s}")
                sB = ps_s.tile([128, 2, 512], f32, tag="s", name=f"sB{s}")
                for i in range(4):
                    m = 4 * t + i
                    dst = (sA if i < 2 else sB)[:, i % 2, :]
                    nc.tensor.matmul(
                        dst,
                        kT[32 * i : 32 * i + 32, 128 * m : 128 * m + 128],
                        qT[32 * i : 32 * i + 32, 512 * g : 512 * g + 512],
                        start=True,
                        stop=True,
                        tile_position=(32 * i, 0),
                    )
                etA = expp.tile([128, 2, 512], fp8e5, tag="e", name=f"eA{s}")
                etB = expp.tile([128, 2, 512], fp8e5, tag="e", name=f"eB{s}")
                nc.scalar.activation(etA[:], sA[:], Exp, bias=negc0[:])
                nc.vector.tensor_scalar(
                    etB[:].bitcast(u8), sB[:], EXP_A, EXP_B, Mult, Add
                )
                sts[s] = (sA, sB)
                ets[s] = (etA, etB)

            def consume(s):
                g, t = divmod(s, NSTEP)
                etA, etB = ets.pop(s)
                sts.pop(s)
                gp = gtile[g]
                dp = dtile[g]
                for pa, et in ((0, etA), (1, etB)):
                    pp = 2 * t + pa
                    for h in range(2):
                        nc.tensor.matmul(
                            gp[:, h, :],
                            x8sb[:, 4 * t + 2 * pa : 4 * t + 2 * pa + 2, 128 * h : 128 * h + 128],
                            et[:],
                            start=(pp == 0),
                            stop=(pp == 2 * NSTEP - 1),
                            perf_mode=DR,
                        )
                for j in range(4):
                    et = (etA if j < 2 else etB)
                    nc.tensor.matmul(
                        dp[32 * j : 32 * j + 32, :],
                        ones8[:],
                        et[:, j % 2, :],
                        start=(t == 0),
                        stop=(t == NSTEP - 1),
                        tile_position=(0, 32 * j),
                    )

            def group_epilogue(g):
                gp = gtile.pop(g)
                dp = dtile.pop(g)
                gsb = small.tile([128, 2, 512], bf, tag="gsb")
                nc.vector.tensor_copy(gsb[:], gp[:])
                dsb = small.tile([128, 512], bf, tag="dsb")
                nc.vector.tensor_copy(dsb[:], dp[:])
                er = ps_e.tile([128, 512], f32, tag="er", name=f"er{g}")
                for b in range(4):
                    nc.tensor.matmul(
                        er[:, 256 + b : 257 + b], dsb[:, 128 * b : 128 * b + 128],
                        ones1[:], start=True, stop=True,
                    )
                rec = small.tile([128, 4], f32, tag="recs")
                nc.vector.reciprocal(rec[:], er[:, 256:260])
                for b in range(4):
                    ep = er[:, 0:256]
                    nc.tensor.matmul(ep, gsb[:, 0, 128 * b : 128 * b + 128], w2sb[:, 0, :], start=True, stop=False)
                    nc.tensor.matmul(ep, gsb[:, 1, 128 * b : 128 * b + 128], w2sb[:, 1, :], start=False, stop=True)
                    qb = 4 * g + b
                    xq = small.tile([128, 256], f32, tag="xq", bufs=3)
                    nc.scalar.dma_start(out=xq[:], in_=xq32_d[128 * qb : 128 * qb + 128, :])
                    ot = small.tile([128, 256], f32, tag="ot", bufs=3)
                    nc.vector.scalar_tensor_tensor(
                        ot[:], ep, rec[:, b : b + 1], xq[:], Mult, Add
                    )
                    nc.sync.dma_start(out=out_d[128 * qb : 128 * qb + 128, :], in_=ot[:])

            for s in range(NG * NSTEP + 2):
                if s < NG * NSTEP:
                    produce(s)
                if s >= 2:
                    sc = s - 2
                    consume(sc)
                    if sc % NSTEP == NSTEP - 1:
                        group_epilogue(sc // NSTEP)

    nc.compile()
    return nc


def _get_compiled():
    if "v2" not in _compiled_cache:
        _compiled_cache["v2"] = _build()
    return _compiled_cache["v2"]


def _prep(x, wq, bq, wk, bk, wv, bv, wo, bo):
    xf = np.ascontiguousarray(np.asarray(x, dtype=np.float32)).reshape(B, N, C)
    wq = np.asarray(wq, np.float32)
    bq = np.asarray(bq, np.float32)
    wk = np.asarray(wk, np.float32)
    bk = np.asarray(bk, np.float32)
    wv = np.asarray(wv, np.float32)
    bv = np.asarray(bv, np.float32)
    wo = np.asarray(wo, np.float32)
    bo = np.asarray(bo, np.float32)

    scale = np.float32(1.0 / np.sqrt(np.float32(D)))
    wqa_rep = np.ascontiguousarray(np.tile(wq * scale, (1, 4))).astype(BF16)  # [256,128]
    wka_rep = np.ascontiguousarray(np.tile(wk, (1, 4))).astype(BF16)
    bq_col = np.ascontiguousarray(np.tile(bq * scale, 4)[:, None]).astype(np.float32)
    bk_col = np.ascontiguousarray(np.tile(bk, 4)[:, None]).astype(np.float32)
    w2 = np.ascontiguousarray(32.0 * (wv @ wo)).astype(BF16)  # [256,256]; x32 folds denom replication
    r0 = (bv @ wo + bo).astype(np.float32)  # residual-folded constant bias row

    in_maps = []
    for core in range(NCORES):
        b, h = divmod(core, 2)
        if h == 0:
            xo = xf[b]
        else:
            xo = np.concatenate([xf[b, NQ:], xf[b, :NQ]], 0)
        in_maps.append(
            {
                "x16": xo.astype(BF16),
                "x8": xo.astype(E4M3),
                "xq32": np.ascontiguousarray(xo[:NQ] + r0[None, :]),
                "wqa_rep": wqa_rep,
                "wka_rep": wka_rep,
                "bq_col": bq_col,
                "bk_col": bk_col,
                "w2": w2,
            }
        )
    return in_maps


def _gather(results):
    out = np.empty((B, N, C), np.float32)
    for core in range(NCORES):
        b, h = divmod(core, 2)
        out[b, NQ * h : NQ * (h + 1)] = results[core]["out"]
    return out.reshape(B, HH, WW, C)


def kernel(x, wq, bq, wk, bk, wv, bv, wo, bo):
    from concourse.bass_utils import run_bass_kernel_spmd

    in_maps = _prep(x, wq, bq, wk, bk, wv, bv, wo, bo)
    nc = _get_compiled()
    res = run_bass_kernel_spmd(nc, in_maps, core_ids=list(range(NCORES)))
    return _gather(res.results)


def _ensure_ntff_hook():
    """The agent image's antenv stub lacks axon_hooks; synthesize it so
    run_bass_kernel_spmd(trace=True) can NTFF-profile via libaxon_pjrt."""
    import types

    try:
        from antenv.axon_hooks import get_axon_ntff_profile_hook  # noqa: F401
        return
    except ImportError:
        pass
    import antenv
    from trn_agent_boot.trn_boot import _ntff_profile_via_ctypes

    mod = types.ModuleType("antenv.axon_hooks")
    state = {"h": _ntff_profile_via_ctypes("/opt/axon/libaxon_pjrt.so")}
    mod.get_axon_ntff_profile_hook = lambda: state["h"]
    mod.set_axon_ntff_profile_hook = lambda h: state.__setitem__("h", h)
    sys.modules["antenv.axon_hooks"] = mod
    antenv.axon_hooks = mod


def run_traced(inputs, **kw):
    """For test.py: run with NTFF profiling; returns (output, BassKernelResults)."""
    from concourse.bass_utils import run_bass_kernel_spmd

    _ensure_ntff_hook()

    in_maps = _prep(**inputs)
    nc = _get_compiled()
    res = run_bass_kernel_spmd(nc, in_maps, core_ids=list(range(NCORES)), trace=True, **kw)
    return _gather(res.results), res
